# revision 19
# baseline (speedup 1.0000x reference)
"""CenterLossLayer Trainium2 kernel — 8-core SPMD, collective-free.

Math (reference):
    sel   = onehot @ centers                      # [B, D] — a row gather
    delta = onehot.T @ (sel - features)           # [C, D] — a scatter-add
    counts = onehot.sum(0) + 1                    # [C, 1]
    new_centers = centers - ALPHA * delta / counts
    loss = sum((features - sel)^2, axis=1)        # [B, 1]

Since row i of `onehot @ centers` is exactly centers[label_i]:
    delta = counts ⊙ centers − onehot.T @ features
    new_centers = centers·s1 + (onehot.T @ features)·s2,
        s1 = (1−ALPHA) + ALPHA/(counts+1),  s2 = ALPHA/(counts+1)

Sharding: pure CLASS sharding — core j owns classes [1250j, 1250j+1250) and
reads the matching onehot COLUMN slice [4096, 1250] (same total onehot
traffic as row sharding) plus the full features. No inter-core
communication at all:
  * delta matmul: lhsT = bf16 cast of the onehot slice (no label decode),
    rhs = [features_bf16 | 1] chunks; the ones column gives per-class
    counts over the whole batch.
  * loss: per 128-row chunk, DVE max_index over the f32 slice (row max is
    known to be 1.0) -> local class index, or huge u32 on miss; sel is
    pre-filled with the row's own bf16 features and a bounds-checked
    indirect gather (4 chunks per instruction) overwrites matched rows
    with centers_l[idx], so unmatched rows give diff == 0 exactly (HW
    leaves OOB rows untouched; CoreSim zero-fills them instead, so sim
    shows a known loss mismatch). Each core emits a full-size partial
    loss vector; the host SUMS the 8 partials.
Engine split: ScalarE casts/squares, VectorE scans/reduces/prefills,
GpSimd gathers+subtracts, PE 320 accumulating matmuls (8 class tiles in 8
PSUM banks pipelined chunk-by-chunk, last 2 tiles as a short tail). Each
matmul group is artificially gated on the bf16 cast 6 chunks ahead
(add_dep_helper) so the PE stream stays dense and HAM-warm instead of
oscillating at the chunk-production rate.
"""
import sys

import numpy as np

sys.path.insert(0, "/opt/trn_rl_repo")

import concourse.bass as bass  # noqa: E402
import concourse.tile as tile  # noqa: E402
from concourse import bacc, mybir  # noqa: E402
from concourse.bass import IndirectOffsetOnAxis  # noqa: E402
from concourse.bass_utils import run_bass_kernel_spmd  # noqa: E402
from concourse.tile import add_dep_helper  # noqa: E402

ALPHA = 0.5
B, C, D = 4096, 10000, 256
N_CORES = 8
CL = C // N_CORES          # 1250 classes per core
P = 128
NGBLK = B // P             # 32 row chunks of 128
EXS = 320                  # rhs chunk stride (pad 257 -> 640B-aligned)
PE_LAG = 6                 # chunks of runway for a dense, HAM-warm PE stream
F32 = mybir.dt.float32
BF16 = mybir.dt.bfloat16
U32 = mybir.dt.uint32
AX = mybir.AxisListType
OP = mybir.AluOpType
AF = mybir.ActivationFunctionType

_CACHE = {}


def _build():
    nc = bacc.Bacc("TRN2", target_bir_lowering=False, debug=False,
                   num_devices=N_CORES)
    oh_cols = nc.dram_tensor("oh_cols", [B, CL], F32,
                             kind="ExternalInput").ap()
    features_full = nc.dram_tensor("features_full", [B, D], F32,
                                   kind="ExternalInput").ap()
    centers_l = nc.dram_tensor("centers_l", [CL, D], F32,
                               kind="ExternalInput").ap()
    loss_p = nc.dram_tensor("loss_p", [B, 1], F32,
                            kind="ExternalOutput").ap()
    newc_l = nc.dram_tensor("newc_l", [CL, D], F32,
                            kind="ExternalOutput").ap()

    with tile.TileContext(nc) as tc:
        with tc.tile_pool(name="const", bufs=1) as constp, \
             tc.tile_pool(name="oh", bufs=4) as ohp, \
             tc.tile_pool(name="big", bufs=1) as bigp, \
             tc.tile_pool(name="f4", bufs=2) as f4p, \
             tc.tile_pool(name="sel", bufs=3) as selp, \
             tc.tile_pool(name="df", bufs=2) as dfp, \
             tc.tile_pool(name="upd", bufs=2) as updp, \
             tc.tile_pool(name="psum", bufs=8, space="PSUM") as psp:

            ones8 = constp.tile([P, 8], F32, name="ones8")
            nc.vector.memset(ones8[:], 1.0)

            rhs_all = bigp.tile([P, NGBLK * EXS], BF16, name="rhs_all")
            recon_all = bigp.tile([P, NGBLK * CL], BF16, name="recon_all")
            idx_all = bigp.tile([P, NGBLK * 8], U32, name="idx_all")
            loss_all = bigp.tile([P, NGBLK], F32, name="loss_all")

            # ---- stage features -> bf16 rhs chunks [feat|1|pad], 4/batch --
            for q in range(8):
                f4 = f4p.tile([P, 4 * D], F32, tag="f4")
                for c in range(4):
                    g = 4 * q + c
                    nc.sync.dma_start(f4[:, c * D:(c + 1) * D],
                                      features_full[g * P:(g + 1) * P, :])
                dst = rhs_all[:, q * 4 * EXS:(q + 1) * 4 * EXS]
                nc.vector.tensor_copy(
                    dst.rearrange("p (n e) -> p n e", n=4)[:, :, 0:D],
                    f4[:].rearrange("p (n d) -> p n d", n=4))
                nc.vector.memset(
                    dst.rearrange("p (n e) -> p n e", n=4)[:, :, D:D + 1],
                    1.0)

            # ---- per-chunk: DMA, bf16 cast (ACT), label scan (DVE) ----
            cast_insts = []
            for g in range(NGBLK):
                oh = ohp.tile([P, CL], F32, tag="oh")
                nc.sync.dma_start(oh[:], oh_cols[g * P:(g + 1) * P, :])
                ci = nc.scalar.activation(
                    out=recon_all[:, g * CL:(g + 1) * CL], in_=oh[:],
                    func=AF.Copy)
                cast_insts.append(ci)
                nc.vector.max_index(idx_all[:, 8 * g:8 * g + 8], ones8[:],
                                    oh[:])

            # ---- loss path, 4 chunks per batch ----
            for q in range(8):
                g0 = 4 * q
                rview = rhs_all[:, g0 * EXS:(g0 + 4) * EXS] \
                    .rearrange("p (n e) -> p n e", n=4)[:, :, 0:D]
                sel4 = selp.tile([P, 4 * D], F32, tag="sel4")
                nc.vector.tensor_copy(
                    sel4[:].rearrange("p (n d) -> p n d", n=4), rview)
                idx4c = selp.tile([P, 4], U32, tag="idx4c")
                nc.vector.tensor_copy(idx4c[:],
                                      idx_all[:, 8 * g0:8 * (g0 + 4):8])
                nc.gpsimd.indirect_dma_start(
                    out=sel4[:].rearrange("p (n d) -> p n d", n=4),
                    out_offset=None, in_=centers_l[:],
                    in_offset=IndirectOffsetOnAxis(ap=idx4c[:], axis=0),
                    bounds_check=CL - 1, oob_is_err=False)
                diff4 = dfp.tile([P, 4 * D], F32, tag="diff4")
                nc.gpsimd.tensor_tensor(
                    out=diff4[:].rearrange("p (n d) -> p n d", n=4),
                    in0=sel4[:].rearrange("p (n d) -> p n d", n=4),
                    in1=rview, op=OP.subtract)
                sq4 = dfp.tile([P, 4 * D], F32, tag="sq4")
                nc.scalar.activation(out=sq4[:], in_=diff4[:], func=AF.Square)
                nc.vector.reduce_sum(
                    loss_all[:, g0:g0 + 4],
                    sq4[:].rearrange("p (n d) -> p n d", n=4), axis=AX.X)
            nc.sync.dma_start(
                loss_p.rearrange("(g p) o -> p g o", p=P)[:, :, 0],
                loss_all[:])

            # ---- delta matmuls: 8 class-tiles pipelined, 2 as tail ----
            mts = [(m0, min(P, CL - m0)) for m0 in range(0, CL, P)]
            ps = [psp.tile([P, D + 1], F32, tag="ps", name=f"ps_{i}")
                  for i in range(len(mts))]

            def mm_group(g, tiles):
                first = True
                for i in tiles:
                    m0, msz = mts[i]
                    mm = nc.tensor.matmul(
                        out=ps[i][:msz, :],
                        lhsT=recon_all[:, g * CL + m0:g * CL + m0 + msz],
                        rhs=rhs_all[:, g * EXS:g * EXS + D + 1],
                        start=(g == 0), stop=(g == NGBLK - 1))
                    if first and g + PE_LAG < NGBLK:
                        add_dep_helper(mm.ins, cast_insts[g + PE_LAG].ins,
                                       reason="PE runway for warm clock")
                    first = False

            for g in range(NGBLK):
                mm_group(g, range(8))
            for g in range(NGBLK):
                mm_group(g, (8, 9))

            # ---- update: newc = centers*s1 + mm*s2 ----
            for i, (m0, msz) in enumerate(mts):
                cnt1 = updp.tile([P, 1], F32, tag="cnt1")
                nc.vector.tensor_scalar_add(cnt1[:msz], ps[i][:msz, D:D + 1],
                                            1.0)
                recip = updp.tile([P, 1], F32, tag="recip")
                nc.vector.reciprocal(recip[:msz], cnt1[:msz])
                s2 = updp.tile([P, 1], F32, tag="s2")
                nc.vector.tensor_scalar_mul(s2[:msz], recip[:msz], ALPHA)
                s1 = updp.tile([P, 1], F32, tag="s1")
                nc.vector.tensor_scalar(out=s1[:msz], in0=recip[:msz],
                                        scalar1=ALPHA, scalar2=1.0 - ALPHA,
                                        op0=OP.mult, op1=OP.add)
                cen = updp.tile([P, D], F32, tag="cen")
                nc.sync.dma_start(cen[:msz], centers_l[m0:m0 + msz, :])
                t1 = updp.tile([P, D], F32, tag="t1")
                nc.scalar.activation(out=t1[:msz], in_=cen[:msz],
                                     func=AF.Copy, scale=s1[:msz, :1])
                t2 = updp.tile([P, D], F32, tag="t2")
                nc.vector.tensor_scalar(out=t2[:msz], in0=ps[i][:msz, 0:D],
                                        scalar1=s2[:msz, :1],
                                        scalar2=None, op0=OP.mult)
                newc = updp.tile([P, D], F32, tag="newc")
                nc.vector.tensor_add(newc[:msz], t1[:msz], t2[:msz])
                nc.sync.dma_start(newc_l[m0:m0 + msz, :], newc[:msz])
    nc.compile()
    return nc


def _get_nc():
    if "nc" not in _CACHE:
        _CACHE["nc"] = _build()
    return _CACHE["nc"]


def _in_maps(features, onehot, centers):
    return [{
        "oh_cols": np.ascontiguousarray(onehot[:, i * CL:(i + 1) * CL]),
        "features_full": features,
        "centers_l": centers[i * CL:(i + 1) * CL],
    } for i in range(N_CORES)]


def kernel(features, onehot, centers):
    features = np.ascontiguousarray(features, dtype=np.float32)
    onehot = np.ascontiguousarray(onehot, dtype=np.float32)
    centers = np.ascontiguousarray(centers, dtype=np.float32)
    nc = _get_nc()
    res = run_bass_kernel_spmd(nc, _in_maps(features, onehot, centers),
                               core_ids=list(range(N_CORES)))
    loss = np.sum([res.results[i]["loss_p"] for i in range(N_CORES)], axis=0)
    new_centers = np.concatenate(
        [res.results[i]["newc_l"] for i in range(N_CORES)], axis=0)
    return loss, new_centers


# revision 20
# speedup vs baseline: 1.4309x; 1.4309x over previous
"""CenterLossLayer Trainium2 kernel — 8-core SPMD, collective-free.

Math (reference):
    sel   = onehot @ centers                      # [B, D] — a row gather
    delta = onehot.T @ (sel - features)           # [C, D] — a scatter-add
    counts = onehot.sum(0) + 1                    # [C, 1]
    new_centers = centers - ALPHA * delta / counts
    loss = sum((features - sel)^2, axis=1)        # [B, 1]

Since row i of `onehot @ centers` is exactly centers[label_i]:
    delta = counts ⊙ centers − onehot.T @ features
    new_centers = centers·s1 + (onehot.T @ features)·s2,
        s1 = (1−ALPHA) + ALPHA/(counts+1),  s2 = ALPHA/(counts+1)

Sharding: pure CLASS sharding — core j owns classes [1250j, 1250j+1250) and
reads the matching onehot COLUMN slice [4096, 1250] (same total onehot
traffic as row sharding) plus the full features. No inter-core
communication at all:
  * delta matmul: lhsT = bf16 cast of the onehot slice (no label decode),
    rhs = [features_bf16 | 1] chunks; the ones column gives per-class
    counts over the whole batch.
  * loss: per 128-row chunk, DVE max_index over the f32 slice (row max is
    known to be 1.0) -> local class index, or huge u32 on miss; sel is
    pre-filled with the row's own bf16 features and a bounds-checked
    indirect gather overwrites matched rows with centers_l[idx], so
    unmatched rows give diff == 0 exactly (HW leaves OOB rows untouched;
    CoreSim zero-fills them instead, so sim shows a known loss mismatch).
    Each core emits a full-size partial loss vector; the host SUMS the 8
    partials (each row is matched by exactly one core).
Engine split: ScalarE casts/squares, VectorE scans/reduces/prefills,
GpSimd gathers+subtracts, PE 320 accumulating matmuls (8 class tiles in 8
PSUM banks pipelined chunk-by-chunk, last 2 tiles as a tail pass over the
resident bf16 slices). The onehot stream owns the HWDGE (sync) queue from
t=0; feature staging, center preloads and gathers ride the SWDGE (gpsimd)
queue so chunk production never stalls.
"""
import sys

import numpy as np

sys.path.insert(0, "/opt/trn_rl_repo")

import concourse.bass as bass  # noqa: E402
import concourse.tile as tile  # noqa: E402
from concourse import bacc, mybir  # noqa: E402
from concourse.bass import IndirectOffsetOnAxis  # noqa: E402
from concourse.bass_utils import run_bass_kernel_spmd  # noqa: E402

ALPHA = 0.5
B, C, D = 4096, 10000, 256
N_CORES = 8
CL = C // N_CORES          # 1250 classes per core
P = 128
NGBLK = B // P             # 32 row chunks of 128
EXS = 320                  # rhs chunk stride (257 used, 640B-aligned)
F32 = mybir.dt.float32
BF16 = mybir.dt.bfloat16
U32 = mybir.dt.uint32
AX = mybir.AxisListType
OP = mybir.AluOpType
AF = mybir.ActivationFunctionType

_CACHE = {}


def _build():
    nc = bacc.Bacc("TRN2", target_bir_lowering=False, debug=False,
                   num_devices=N_CORES)
    oh_cols = nc.dram_tensor("oh_cols", [B, CL], F32,
                             kind="ExternalInput").ap()
    features_full = nc.dram_tensor("features_full", [B, D], F32,
                                   kind="ExternalInput").ap()
    centers_l = nc.dram_tensor("centers_l", [CL, D], F32,
                               kind="ExternalInput").ap()
    loss_p = nc.dram_tensor("loss_p", [B, 1], F32,
                            kind="ExternalOutput").ap()
    newc_l = nc.dram_tensor("newc_l", [CL, D], F32,
                            kind="ExternalOutput").ap()

    with tile.TileContext(nc) as tc:
        with tc.tile_pool(name="const", bufs=1) as constp, \
             tc.tile_pool(name="oh", bufs=5) as ohp, \
             tc.tile_pool(name="big", bufs=1) as bigp, \
             tc.tile_pool(name="f4", bufs=2) as f4p, \
             tc.tile_pool(name="sel", bufs=6) as selp, \
             tc.tile_pool(name="df", bufs=4) as dfp, \
             tc.tile_pool(name="upd", bufs=4) as updp, \
             tc.tile_pool(name="psum", bufs=8, space="PSUM") as psp:

            ones8 = constp.tile([P, 8], F32, name="ones8")
            nc.vector.memset(ones8[:], 1.0)

            rhs_all = bigp.tile([P, NGBLK * EXS], BF16, name="rhs_all")
            recon_all = bigp.tile([P, NGBLK * CL], BF16, name="recon_all")
            idx_all = bigp.tile([P, NGBLK * 8], U32, name="idx_all")
            loss_all = bigp.tile([P, NGBLK], F32, name="loss_all")
            cen_all = bigp.tile([P, 10 * D], F32, name="cen_all")

            # ---- preloads on the SWDGE queue: centers + features ----
            mts = [(m0, min(P, CL - m0)) for m0 in range(0, CL, P)]
            for i, (m0, msz) in enumerate(mts):
                nc.gpsimd.dma_start(cen_all[:msz, i * D:(i + 1) * D],
                                    centers_l[m0:m0 + msz, :])
            for q in range(8):
                f4 = f4p.tile([P, 4 * D], F32, tag="f4")
                for c in range(4):
                    g = 4 * q + c
                    nc.gpsimd.dma_start(f4[:, c * D:(c + 1) * D],
                                        features_full[g * P:(g + 1) * P, :])
                dst = rhs_all[:, q * 4 * EXS:(q + 1) * 4 * EXS]
                nc.vector.tensor_copy(
                    dst.rearrange("p (n e) -> p n e", n=4)[:, :, 0:D],
                    f4[:].rearrange("p (n d) -> p n d", n=4))
                nc.vector.memset(
                    dst.rearrange("p (n e) -> p n e", n=4)[:, :, D:D + 1],
                    1.0)

            # ---- per-chunk: oh DMA (sync queue), bf16 cast, scan, loss ----
            for g in range(NGBLK):
                oh = ohp.tile([P, CL], F32, tag="oh")
                nc.sync.dma_start(oh[:], oh_cols[g * P:(g + 1) * P, :])
                nc.scalar.activation(
                    out=recon_all[:, g * CL:(g + 1) * CL], in_=oh[:],
                    func=AF.Copy)
                nc.vector.max_index(idx_all[:, 8 * g:8 * g + 8], ones8[:],
                                    oh[:])

            for g in range(NGBLK):
                fsl = rhs_all[:, g * EXS:g * EXS + D]
                sel = selp.tile([P, D], F32, tag="sel")
                nc.vector.tensor_copy(sel[:], fsl)
                nc.gpsimd.indirect_dma_start(
                    out=sel[:], out_offset=None, in_=centers_l[:],
                    in_offset=IndirectOffsetOnAxis(
                        ap=idx_all[:, 8 * g:8 * g + 1], axis=0),
                    bounds_check=CL - 1, oob_is_err=False)
                diff = dfp.tile([P, D], F32, tag="diff")
                nc.gpsimd.tensor_sub(diff[:], sel[:], fsl)
                sq = dfp.tile([P, D], F32, tag="sq")
                nc.scalar.activation(out=sq[:], in_=diff[:], func=AF.Square)
                nc.vector.reduce_sum(loss_all[:, g:g + 1], sq[:], axis=AX.X)
            nc.sync.dma_start(
                loss_p.rearrange("(g p) o -> p g o", p=P)[:, :, 0],
                loss_all[:])

            # ---- delta matmuls: 8 class-tiles pipelined, 2 as tail ----
            ps = [psp.tile([P, D + 1], F32, tag="ps", name=f"ps_{i}")
                  for i in range(len(mts))]

            def mm_group(g, tiles):
                for i in tiles:
                    m0, msz = mts[i]
                    nc.tensor.matmul(
                        out=ps[i][:msz, :],
                        lhsT=recon_all[:, g * CL + m0:g * CL + m0 + msz],
                        rhs=rhs_all[:, g * EXS:g * EXS + D + 1],
                        start=(g == 0), stop=(g == NGBLK - 1))

            for g in range(NGBLK):
                mm_group(g, range(8))
            for g in range(NGBLK):
                mm_group(g, (8, 9))

            # ---- update: newc = centers*s1 + mm*s2 ----
            for i, (m0, msz) in enumerate(mts):
                cnt1 = updp.tile([P, 1], F32, tag="cnt1")
                nc.vector.tensor_scalar_add(cnt1[:msz], ps[i][:msz, D:D + 1],
                                            1.0)
                recip = updp.tile([P, 1], F32, tag="recip")
                nc.vector.reciprocal(recip[:msz], cnt1[:msz])
                s2 = updp.tile([P, 1], F32, tag="s2")
                nc.vector.tensor_scalar_mul(s2[:msz], recip[:msz], ALPHA)
                s1 = updp.tile([P, 1], F32, tag="s1")
                nc.vector.tensor_scalar(out=s1[:msz], in0=recip[:msz],
                                        scalar1=ALPHA, scalar2=1.0 - ALPHA,
                                        op0=OP.mult, op1=OP.add)
                t1 = updp.tile([P, D], F32, tag="t1")
                nc.scalar.activation(out=t1[:msz],
                                     in_=cen_all[:msz, i * D:(i + 1) * D],
                                     func=AF.Copy, scale=s1[:msz, :1])
                t2 = updp.tile([P, D], F32, tag="t2")
                nc.vector.tensor_scalar(out=t2[:msz], in0=ps[i][:msz, 0:D],
                                        scalar1=s2[:msz, :1],
                                        scalar2=None, op0=OP.mult)
                newc = updp.tile([P, D], F32, tag="newc")
                nc.vector.tensor_add(newc[:msz], t1[:msz], t2[:msz])
                nc.sync.dma_start(newc_l[m0:m0 + msz, :], newc[:msz])
    nc.compile()
    return nc


def _get_nc():
    if "nc" not in _CACHE:
        _CACHE["nc"] = _build()
    return _CACHE["nc"]


def _in_maps(features, onehot, centers):
    return [{
        "oh_cols": np.ascontiguousarray(onehot[:, i * CL:(i + 1) * CL]),
        "features_full": features,
        "centers_l": centers[i * CL:(i + 1) * CL],
    } for i in range(N_CORES)]


def kernel(features, onehot, centers):
    features = np.ascontiguousarray(features, dtype=np.float32)
    onehot = np.ascontiguousarray(onehot, dtype=np.float32)
    centers = np.ascontiguousarray(centers, dtype=np.float32)
    nc = _get_nc()
    res = run_bass_kernel_spmd(nc, _in_maps(features, onehot, centers),
                               core_ids=list(range(N_CORES)))
    loss = np.sum([res.results[i]["loss_p"] for i in range(N_CORES)], axis=0)
    new_centers = np.concatenate(
        [res.results[i]["newc_l"] for i in range(N_CORES)], axis=0)
    return loss, new_centers
